# revision 17
# baseline (speedup 1.0000x reference)
# DeepSeek-style MoE PINN kernel for Trainium2 (Bass/Tile), 8-core data parallel.
#
# Math (per reference):
#   h = tanh([x,t] @ W_in + b_in)                        [N,64]
#   3x MoE layer:
#     sh = tanh(h @ sW1[e] + sb1[e]) for 2 shared experts
#     shared_out = sum_e sh[e] @ sW2[e] + sb2.sum(0)
#     logits = h @ routW + routb ; rw = softmax(logits)
#     top-2 mask (not renormalized): w = rw * mask
#     rh = tanh(h @ rW1[e] + rb1[e]) for 4 routed experts
#     r_out[e] = rh[e] @ rW2[e] + rb2[e]
#     routed_out = sum_e w[:,e] * r_out[e]
#     h = tanh(h + shared_out + routed_out)
#   y = h @ W_out + b_out
#
# Two variants are built:
#  - "fast": requires all biases (b_in, sb1, rb1, sb2, routb, b_out) and rb2
#    to be zero (true for the reference's setup_inputs).  bf16 matmuls with
#    paired-tile layout h[128, tok/2] (two 512-token tiles on partition
#    halves), fused [128,1536] tanh, residual folded into PSUM via identity
#    matmul, router scale broadcast via one DMA-transpose + one DRAM bounce
#    write + two broadcast reads per 4096-token block.
#  - "general": the original f32 implementation (correct for any inputs).

import numpy as np

N_TOTAL = 262144
D = 64
L = 3
NCORES = 8
NPC = N_TOTAL // NCORES  # tokens per core (32768)
MB = 8192                # tokens per macro-batch (h ping/pong resident)
NMB = NPC // MB          # 4
TT = 512                 # tokens per tile
TPM = MB // TT           # 16 tiles per macro-batch
GRP = 4                  # tiles per router group (general variant)
NG = TPM // GRP          # router groups per macro-batch

# fast variant geometry
PT = 1024                # tokens per pairtile (2 tiles packed on partitions)
PPM = MB // PT           # 8 pairtiles per macro-batch
RBT = 8                  # tiles per router block (= 4 pairtiles = 4096 tokens)
NRB = TPM // RBT         # 2 router blocks per macro-batch

_CACHE = {}


def _build_fast_module(npc: int = NPC, mbsz: int = MB, ncores: int = NCORES):
    NPC = npc
    MB = mbsz
    NMB = NPC // MB
    PPM = MB // PT
    NRB = (MB // TT) // RBT
    from contextlib import ExitStack

    import concourse.bass as bass
    import concourse.tile as tile
    from concourse import bacc, mybir

    f32 = mybir.dt.float32
    f32r = mybir.dt.float32r
    bf16 = mybir.dt.bfloat16
    AF = mybir.ActivationFunctionType
    OP = mybir.AluOpType

    nc = bacc.Bacc("TRN2", num_devices=ncores, debug=False, enable_asserts=False)

    xin = nc.dram_tensor("xin", [2, NPC], f32, kind="ExternalInput").ap()
    w1c = nc.dram_tensor("w1c", [L, 128, 768], f32, kind="ExternalInput").ap()
    w2c = nc.dram_tensor("w2c", [L, 128, 192], bf16, kind="ExternalInput").ap()
    rtc = nc.dram_tensor("rtc", [L, 128, 8], f32, kind="ExternalInput").ap()
    winc = nc.dram_tensor("winc", [2, 256], f32, kind="ExternalInput").ap()
    woutc = nc.dram_tensor("woutc", [128, 512], f32, kind="ExternalInput").ap()
    y = nc.dram_tensor("y", [NPC, 1], f32, kind="ExternalOutput").ap()
    yv = y.rearrange("(a b) o -> a (b o)", b=4 * TT)  # head output rows [16,2048]

    with tile.TileContext(nc) as tc, ExitStack() as ctx:
        singles = ctx.enter_context(tc.tile_pool(name="singles", bufs=1))
        hpool = ctx.enter_context(tc.tile_pool(name="hpool", bufs=2))
        xtpool = ctx.enter_context(tc.tile_pool(name="xtpool", bufs=2))
        shpool = ctx.enter_context(tc.tile_pool(name="shpool", bufs=4))
        rscpool = ctx.enter_context(tc.tile_pool(name="rscpool", bufs=4))
        hapool = ctx.enter_context(tc.tile_pool(name="hapool", bufs=3))
        rout = ctx.enter_context(tc.tile_pool(name="rout", bufs=3))
        pwpool = ctx.enter_context(tc.tile_pool(name="pwpool", bufs=4))
        yspool = ctx.enter_context(tc.tile_pool(name="yspool", bufs=2))
        ps = ctx.enter_context(tc.tile_pool(name="ps", bufs=1, space="PSUM"))
        dpool = ctx.enter_context(tc.tile_pool(name="dpool", bufs=3, space="DRAM"))

        # --- constants to SBUF (once); gpsimd cast-DMA rounds f32 -> f32r ---
        w1_sb = []
        w2_sb = []
        rt_sb = []
        for l in range(L):
            wl = singles.tile([128, 768], f32r, name=f"w1l{l}")
            nc.gpsimd.dma_start(out=wl, in_=w1c[l])
            w1_sb.append(wl)
            w2l = singles.tile([128, 192], bf16, name=f"w2l{l}")
            nc.sync.dma_start(out=w2l, in_=w2c[l])
            w2_sb.append(w2l)
            rtl = singles.tile([128, 8], f32r, name=f"rtl{l}")
            nc.gpsimd.dma_start(out=rtl, in_=rtc[l])
            rt_sb.append(rtl)
        win_sb = singles.tile([2, 256], f32r)
        nc.gpsimd.dma_start(out=win_sb, in_=winc)
        wout_sb = singles.tile([128, 512], f32r)
        nc.gpsimd.dma_start(out=wout_sb, in_=woutc)

        for mb in range(NMB):
            mbs = slice(mb * MB, (mb + 1) * MB)
            xt = xtpool.tile([2, MB], f32r, tag="xt")
            nc.gpsimd.dma_start(out=xt, in_=xin[:, mbs])

            # ---- layer 0: h2[:, pt*512:+512] = tanh(W_in^T @ [x;t]) paired ----
            h2 = hpool.tile([128, MB // 2], f32r, tag="h")
            for pt in range(PPM):
                p0 = ps.tile([128, TT], f32, tag="po", bufs=1)
                nc.tensor.matmul(
                    p0, win_sb[:, 0:128], xt[:, pt * PT : pt * PT + TT],
                    start=True, stop=False,
                )
                nc.tensor.matmul(
                    p0, win_sb[:, 128:256], xt[:, pt * PT + TT : (pt + 1) * PT],
                    start=False, stop=True,
                )
                nc.scalar.activation(
                    h2[:, pt * TT : (pt + 1) * TT], p0, AF.Tanh
                )

            # ---- MoE layers ----
            for l in range(L):
                hn = hpool.tile([128, MB // 2], f32r, tag="h")
                pws_all = []
                for rb in range(NRB):
                    # ===== router for pairtiles [4rb, 4rb+4): token-major =====
                    plg = ps.tile([128, 128], f32, tag="rt", bufs=1)
                    for k in range(4):
                        pt = rb * 4 + k
                        for cl in range(4):
                            hc = h2[:, pt * TT + cl * 128 : pt * TT + (cl + 1) * 128]
                            nc.tensor.matmul(
                                plg[:, (k * 4 + cl) * 8 : (k * 4 + cl + 1) * 8],
                                hc,
                                rt_sb[l][:, :],
                                start=True,
                                stop=True,
                            )
                    eeg = rout.tile([128, 128], f32, tag="ee")
                    nc.scalar.activation(eeg, plg, AF.Exp)
                    e3 = eeg.rearrange("p (q e) -> p q e", e=4)
                    ss = rout.tile([128, 32], f32, tag="ss")
                    nc.vector.reduce_sum(ss, e3, axis=mybir.AxisListType.X)
                    rs = rout.tile([128, 32], f32, tag="rs")
                    nc.vector.reciprocal(rs, ss)
                    m1 = rout.tile([128, 32], f32, tag="m1")
                    nc.vector.tensor_tensor(m1, e3[:, :, 0], e3[:, :, 1], op=OP.max)
                    n1 = rout.tile([128, 32], f32, tag="n1")
                    nc.vector.tensor_tensor(n1, e3[:, :, 0], e3[:, :, 1], op=OP.min)
                    m2 = rout.tile([128, 32], f32, tag="m2")
                    nc.vector.tensor_tensor(m2, e3[:, :, 2], e3[:, :, 3], op=OP.max)
                    n2 = rout.tile([128, 32], f32, tag="n2")
                    nc.vector.tensor_tensor(n2, e3[:, :, 2], e3[:, :, 3], op=OP.min)
                    t1 = rout.tile([128, 32], f32, tag="t1")
                    nc.vector.tensor_tensor(t1, m1, m2, op=OP.min)
                    t2 = rout.tile([128, 32], f32, tag="t2")
                    nc.vector.tensor_tensor(t2, n1, n2, op=OP.max)
                    snd = rout.tile([128, 32], f32, tag="snd")
                    nc.vector.tensor_tensor(snd, t1, t2, op=OP.max)
                    mk = rout.tile([128, 128], f32, tag="mk")
                    snd_b = snd.unsqueeze(2).broadcast_to((128, 32, 4))
                    nc.vector.tensor_tensor(
                        mk.rearrange("p (q e) -> p q e", e=4), e3, snd_b, op=OP.is_ge
                    )
                    wu = rout.tile([128, 128], f32, tag="wu")
                    nc.vector.tensor_mul(wu, eeg, mk)
                    # wf columns reordered to (tg, e, cl) so the transpose's
                    # partition order directly gives the DRAM bounce layout
                    # (rows (tg,e) of 512 contiguous tokens).
                    wf = rout.tile([128, 128], bf16, tag="wf")
                    wf_v = bass.AP(
                        tensor=wf.tensor, offset=wf.offset,
                        ap=[list(p) for p in wf[:, 0:1].ap[:1]]
                        + [[32, 4], [1, 4], [16, 2], [4, 4]],
                    )
                    wu_v = bass.AP(
                        tensor=wu.tensor, offset=wu.offset,
                        ap=[list(p) for p in wu[:, 0:1].ap[:1]]
                        + [[32, 4], [8, 4], [4, 2], [1, 4]],
                    )
                    rs_v = bass.AP(
                        tensor=rs.tensor, offset=rs.offset,
                        ap=[list(p) for p in rs[:, 0:1].ap[:1]]
                        + [[8, 4], [2, 4], [1, 2], [0, 4]],
                    )
                    nc.vector.tensor_mul(wf_v, wu_v, rs_v)
                    # transpose wf [128,(tg e cl)] -> [(tg e cl), 128] via the
                    # XBAR DMA transpose (bf16), bounce through DRAM, then
                    # broadcast-read expert rows into [128, tokens] scale tiles.
                    wtt = rout.tile([128, 128], bf16, tag="wtt")
                    nc.sync.dma_start(out=wtt, in_=wf, transpose=True)
                    wdr = dpool.tile([32, 512], bf16, tag="wdr")
                    nc.sync.dma_start(
                        out=wdr.rearrange("a (b t) -> (a b) t", t=128), in_=wtt
                    )
                    # broadcast reads: half 0 = experts {0,2}, half 1 = {1,3};
                    # (tg, pair) collapses to one uniform-stride dim of 16.
                    w0 = wdr[0:1, 0:1]
                    pws = []
                    for sb in range(2):
                        pwsb = pwpool.tile([128, RBT * PT // 2], bf16, tag="pw")
                        pws.append(pwsb)
                        for hh in range(2):
                            nc.sync.dma_start(
                                out=pwsb[64 * hh : 64 * hh + 64, :].rearrange(
                                    "p (b t) -> p b t", t=TT
                                ),
                                in_=bass.AP(
                                    tensor=w0.tensor,
                                    offset=w0.offset + hh * TT + sb * 8192,
                                    ap=[[0, 64], [1024, 8], [1, TT]],
                                ),
                            )

                    pws_all.append(pws)
                for rb in range(NRB):
                    pws = pws_all[rb]
                    # ===== main per-pairtile compute =====
                    for ptb in range(4):
                        pt = rb * 4 + ptb
                        ptsl = bass.ts(pt, TT)  # h2 col slice for this pairtile
                        hs = h2[:, ptsl]
                        pwsb = pws[ptb // 2]
                        po = ps.tile([128, TT], f32, tag="po", bufs=1)
                        for hf in range(2):
                            psw1 = ps.tile([128, 3 * TT], f32, tag="w1", bufs=2)
                            for j in range(3):
                                nc.tensor.matmul(
                                    psw1[:, j * TT : (j + 1) * TT],
                                    w1_sb[l][:, (2 * j + hf) * 128
                                             : (2 * j + hf + 1) * 128],
                                    hs, start=True, stop=True,
                                )
                            sh = shpool.tile([128, 3 * TT], bf16, tag="sh")
                            nc.scalar.activation(sh, psw1, AF.Tanh)
                            rsc = rscpool.tile([128, 2 * TT], bf16, tag="rsc")
                            pwofs = (2 * (ptb % 2) + hf) * PT
                            nc.vector.tensor_mul(
                                rsc, sh[:, TT : 3 * TT],
                                pwsb[:, pwofs : pwofs + 2 * TT],
                            )
                            pos = po[64 * hf : 64 * hf + 64, :]
                            for j, mv in enumerate((
                                sh[:, 0:TT], rsc[:, 0:TT], rsc[:, TT : 2 * TT],
                            )):
                                nc.tensor.matmul(
                                    pos, w2_sb[l][:, j * 64 : (j + 1) * 64],
                                    mv, start=(j == 0), stop=(j == 2),
                                )
                        ha = hapool.tile([128, TT], f32, tag="ha")
                        nc.vector.tensor_add(ha, po, h2[:, ptsl].bitcast(f32))
                        nc.scalar.activation(hn[:, ptsl], ha, AF.Tanh)
                h2 = hn

            # ---- head: 2 pairtiles (4 tiles) per PSUM tile ----
            for hg in range(PPM // 2):
                py = ps.tile([128, TT], f32, tag="po", bufs=1)
                for k in range(4):
                    pt = hg * 2 + k // 2
                    nc.tensor.matmul(
                        py,
                        wout_sb[:, k * 128 : (k + 1) * 128],
                        h2[:, bass.ts(pt, TT)],
                        start=(k == 0), stop=(k == 3),
                    )
                ysb = yspool.tile([128, TT], f32, tag="ysb")
                nc.vector.tensor_copy(ysb, py)
                yrow = ysb.rearrange("(a b) f -> a b f", b=32)[:, 0, :]  # [4, TT]
                nc.sync.dma_start(
                    out=yv[mb * (PPM // 2) + hg : mb * (PPM // 2) + hg + 1, :],
                    in_=yrow,
                )

    nc.compile()
    return nc


def _build_module(with_rb2: bool, npc: int = NPC, mbsz: int = MB, ncores: int = NCORES):
    NPC = npc
    MB = mbsz
    NMB = NPC // MB
    TPM = MB // TT
    NG = TPM // GRP
    from contextlib import ExitStack

    import concourse.bass as bass
    import concourse.tile as tile
    from concourse import bacc, mybir

    f32 = mybir.dt.float32
    AF = mybir.ActivationFunctionType
    OP = mybir.AluOpType

    nc = bacc.Bacc("TRN2", num_devices=ncores, debug=False, enable_asserts=False)

    xin = nc.dram_tensor("xin", [2, NPC], f32, kind="ExternalInput").ap()
    w1c = nc.dram_tensor("w1c", [L, 64, 384], f32, kind="ExternalInput").ap()
    w2c = nc.dram_tensor("w2c", [L, 128, 768], f32, kind="ExternalInput").ap()
    rtc = nc.dram_tensor("rtc", [L, 64, 4], f32, kind="ExternalInput").ap()
    rb2c = nc.dram_tensor("rb2c", [L, 2, 128, 64], f32, kind="ExternalInput").ap()
    # bias columns: 0: b_in; 1+3l: sb1[l]; 2+3l: rb1[l,0:2]; 3+3l: rb1[l,2:4];
    # 10+l: sb2[l].sum(0); 13: b_out (replicated)
    bcl = nc.dram_tensor("bcl", [128, 14], f32, kind="ExternalInput").ap()
    # exp(routb[l]) replicated over partitions
    crw = nc.dram_tensor("crw", [128, 4 * L], f32, kind="ExternalInput").ap()
    winc = nc.dram_tensor("winc", [2, 64], f32, kind="ExternalInput").ap()
    woutc = nc.dram_tensor("woutc", [64, 32], f32, kind="ExternalInput").ap()
    idc = nc.dram_tensor("idc", [128, 128], f32, kind="ExternalInput").ap()
    y = nc.dram_tensor("y", [NPC, 1], f32, kind="ExternalOutput").ap()
    yv = y.rearrange("(a b) o -> a (b o)", b=min(4 * TT, NPC))  # head output rows

    with tile.TileContext(nc) as tc, ExitStack() as ctx:
        singles = ctx.enter_context(tc.tile_pool(name="singles", bufs=1))
        hpool = ctx.enter_context(tc.tile_pool(name="hpool", bufs=2))
        xtpool = ctx.enter_context(tc.tile_pool(name="xtpool", bufs=2))
        stage = ctx.enter_context(tc.tile_pool(name="stage", bufs=3))
        rstage = ctx.enter_context(tc.tile_pool(name="rstage", bufs=2))
        small = ctx.enter_context(tc.tile_pool(name="small", bufs=4))
        ps = ctx.enter_context(tc.tile_pool(name="ps", bufs=1, space="PSUM"))
        dpool = ctx.enter_context(tc.tile_pool(name="dpool", bufs=4, space="DRAM"))

        # --- constants to SBUF (once) ---
        w1_sb = []
        w2_sb = []
        rt_sb = []
        rb2_sb = []
        for l in range(L):
            wl = singles.tile([64, 384], f32, name=f"w1l{l}")
            nc.sync.dma_start(out=wl, in_=w1c[l])
            w1_sb.append(wl)
            w2l = singles.tile([128, 192], f32, name=f"w2l{l}")
            nc.sync.dma_start(out=w2l, in_=w2c[l])
            w2_sb.append(w2l)
            rtl = singles.tile([64, 4], f32, name=f"rtl{l}")
            nc.sync.dma_start(out=rtl, in_=rtc[l])
            rt_sb.append(rtl)
            if with_rb2:
                rbl = singles.tile([128, 2, 64], f32, name=f"rbl{l}")
                nc.sync.dma_start(
                    out=rbl, in_=rb2c[l].rearrange("a p f -> p a f")
                )
                rb2_sb.append(rbl)
        bcl_sb = singles.tile([128, 14], f32)
        nc.sync.dma_start(out=bcl_sb, in_=bcl)
        crw_sb = singles.tile([128, 4 * L], f32)
        nc.sync.dma_start(out=crw_sb, in_=crw)
        win_sb = singles.tile([2, 64], f32)
        nc.sync.dma_start(out=win_sb, in_=winc)
        wout_sb = singles.tile([64, 32], f32)
        nc.sync.dma_start(out=wout_sb, in_=woutc)
        id_sb = singles.tile([128, 128], f32)
        nc.sync.dma_start(out=id_sb, in_=idc)

        for mb in range(NMB):
            mbs = slice(mb * MB, (mb + 1) * MB)
            xt = xtpool.tile([2, MB], f32, tag="xt")
            nc.sync.dma_start(out=xt, in_=xin[:, mbs])

            # ---- layer 0: h = tanh(W_in^T @ [x;t] + b_in) ----
            h = hpool.tile([64, MB], f32, tag="h")
            for t in range(TPM):
                tsl = bass.ts(t, TT)
                p0 = ps.tile([64, TT], f32, tag="po", bufs=1, padded_shape=[128, TT])
                nc.tensor.matmul(p0, win_sb, xt[:, tsl], start=True, stop=True)
                nc.scalar.activation(
                    h[:, tsl], p0, AF.Tanh, bias=bcl_sb[0:64, 0:1]
                )

            # ---- MoE layers ----
            for l in range(L):
                hn = hpool.tile([64, MB], f32, tag="h")
                for g in range(NG):
                    # ===== router for tiles [4g, 4g+4): token-major =====
                    plg = ps.tile([128, GRP * 16], f32, tag="lgwt", bufs=2,
                                  padded_shape=[128, TT])
                    for tg in range(GRP):
                        t = g * GRP + tg
                        for c in range(4):
                            hc = h[:, t * TT + c * 128 : t * TT + (c + 1) * 128]
                            nc.tensor.matmul(
                                plg[:, tg * 16 + c * 4 : tg * 16 + (c + 1) * 4],
                                hc,
                                rt_sb[l][:, :],
                                start=True,
                                stop=True,
                            )
                    ee = rstage.tile([128, GRP * 16], f32, tag="ee")
                    nc.scalar.activation(ee, plg, AF.Exp)
                    # multiply by exp(routb) and sum over experts
                    e3 = ee.rearrange("p (q e) -> p q e", e=4)
                    crw_b = (
                        crw_sb[:, 4 * l : 4 * l + 4]
                        .unsqueeze(1)
                        .broadcast_to((128, GRP * 4, 4))
                    )
                    ec = rstage.tile([128, GRP * 16], f32, tag="ec")
                    ec3 = ec.rearrange("p (q e) -> p q e", e=4)
                    nc.vector.tensor_mul(ec3, e3, crw_b)
                    ss = small.tile([128, GRP * 4], f32, tag="ss")
                    nc.vector.reduce_sum(ss, ec3, axis=mybir.AxisListType.X)
                    rs = small.tile([128, GRP * 4], f32, tag="rs")
                    nc.vector.reciprocal(rs, ss)
                    rw = rstage.tile([128, GRP * 16], f32, tag="rw")
                    rs_b = rs.unsqueeze(2).broadcast_to((128, GRP * 4, 4))
                    r3 = rw.rearrange("p (q e) -> p q e", e=4)
                    nc.vector.tensor_mul(r3, ec3, rs_b)
                    m1 = small.tile([128, GRP * 4], f32, tag="m1")
                    nc.vector.tensor_tensor(m1, r3[:, :, 0], r3[:, :, 1], op=OP.max)
                    n1 = small.tile([128, GRP * 4], f32, tag="n1")
                    nc.vector.tensor_tensor(n1, r3[:, :, 0], r3[:, :, 1], op=OP.min)
                    m2 = small.tile([128, GRP * 4], f32, tag="m2")
                    nc.vector.tensor_tensor(m2, r3[:, :, 2], r3[:, :, 3], op=OP.max)
                    n2 = small.tile([128, GRP * 4], f32, tag="n2")
                    nc.vector.tensor_tensor(n2, r3[:, :, 2], r3[:, :, 3], op=OP.min)
                    t1 = small.tile([128, GRP * 4], f32, tag="t1")
                    nc.vector.tensor_tensor(t1, m1, m2, op=OP.min)
                    t2 = small.tile([128, GRP * 4], f32, tag="t2")
                    nc.vector.tensor_tensor(t2, n1, n2, op=OP.max)
                    snd = small.tile([128, GRP * 4], f32, tag="snd")
                    nc.vector.tensor_tensor(snd, t1, t2, op=OP.max)
                    mk = rstage.tile([128, GRP * 16], f32, tag="mk")
                    snd_b = snd.unsqueeze(2).broadcast_to((128, GRP * 4, 4))
                    nc.vector.tensor_tensor(
                        mk.rearrange("p (q e) -> p q e", e=4), r3, snd_b, op=OP.is_ge
                    )
                    wf = rstage.tile([128, GRP * 16], f32, tag="wf")
                    nc.vector.tensor_mul(wf, rw, mk)

                    # ===== main per-tile compute =====
                    for tg in range(GRP):
                        t = g * GRP + tg
                        tsl = bass.ts(t, TT)
                        hs = h[:, tsl]
                        # W1 stage: 2 shared + 4 routed first-layer matmuls
                        psh = ps.tile([128, TT], f32, tag="w1", bufs=5)
                        nc.tensor.matmul(
                            psh, w1_sb[l][:, 0:128], hs, start=True, stop=True
                        )
                        pr1 = ps.tile([128, TT], f32, tag="w1", bufs=5)
                        nc.tensor.matmul(
                            pr1, w1_sb[l][:, 128:256], hs, start=True, stop=True
                        )
                        pr2 = ps.tile([128, TT], f32, tag="w1", bufs=5)
                        nc.tensor.matmul(
                            pr2, w1_sb[l][:, 256:384], hs, start=True, stop=True
                        )
                        sh = stage.tile([128, 3 * TT], f32, tag="sh")
                        nc.scalar.activation(
                            sh[:, 0:TT], psh, AF.Tanh,
                            bias=bcl_sb[:, 1 + 3 * l : 2 + 3 * l],
                        )
                        nc.scalar.activation(
                            sh[:, TT : 2 * TT], pr1, AF.Tanh,
                            bias=bcl_sb[:, 2 + 3 * l : 3 + 3 * l],
                        )
                        nc.scalar.activation(
                            sh[:, 2 * TT : 3 * TT], pr2, AF.Tanh,
                            bias=bcl_sb[:, 3 + 3 * l : 4 + 3 * l],
                        )

                        # transpose masked weights [128,16] -> [16,128]:
                        # row (4c+e) = chunk c's 128 tokens for expert e
                        pwt = ps.tile([16, 128], f32, tag="lgwt", bufs=2,
                                      padded_shape=[128, TT])
                        nc.tensor.transpose(
                            pwt, wf[:, tg * 16 : (tg + 1) * 16], id_sb
                        )
                        wts = rstage.tile([16, 128], f32, tag="wts")
                        nc.vector.tensor_copy(wts, pwt)
                        # bounce w rows through DRAM, then broadcast-read with
                        # step-0 partition + chunk-strided APs (1 DMA / expert)
                        wdr = dpool.tile([16, 128], f32, tag="wdr")
                        nc.sync.dma_start(out=wdr, in_=wts)
                        pwsb = stage.tile([128, 2 * TT], f32, tag="pwsb")
                        def _brd(e):
                            ap = wdr[0:1, 0:1]
                            return bass.AP(
                                tensor=ap.tensor,
                                offset=ap.offset + e * 128,
                                ap=[[0, 64], [512, 4], [1, 128]],
                            )
                        nc.sync.dma_start(out=pwsb[0:64, 0:TT], in_=_brd(0))
                        nc.sync.dma_start(out=pwsb[64:128, 0:TT], in_=_brd(1))
                        nc.sync.dma_start(out=pwsb[0:64, TT : 2 * TT], in_=_brd(2))
                        nc.sync.dma_start(out=pwsb[64:128, TT : 2 * TT], in_=_brd(3))
                        rsc = stage.tile([128, 2 * TT], f32, tag="rsc")
                        nc.vector.tensor_mul(rsc, sh[:, TT : 3 * TT], pwsb)

                        # W2 stage: accumulate shared + routed (+ rb2)
                        po = ps.tile([64, TT], f32, tag="po", bufs=1,
                                     padded_shape=[128, TT])
                        nc.tensor.matmul(
                            po, w2_sb[l][:, 0:64], sh[:, 0:TT],
                            start=True, stop=False,
                        )
                        nc.tensor.matmul(
                            po, w2_sb[l][:, 64:128], rsc[:, 0:TT],
                            start=False, stop=False,
                        )
                        nc.tensor.matmul(
                            po, w2_sb[l][:, 128:192], rsc[:, TT : 2 * TT],
                            start=False, stop=not with_rb2,
                        )
                        if with_rb2:
                            nc.tensor.matmul(
                                po, rb2_sb[l][:, 0, :], pwsb[:, 0:TT],
                                start=False, stop=False,
                            )
                            nc.tensor.matmul(
                                po, rb2_sb[l][:, 1, :], pwsb[:, TT : 2 * TT],
                                start=False, stop=True,
                            )
                        # residual add on DVE, then tanh
                        ha = stage.tile([64, TT], f32, tag="ha")
                        nc.vector.tensor_add(ha, po, h[:, tsl])
                        nc.scalar.activation(
                            hn[:, tsl], ha, AF.Tanh,
                            bias=bcl_sb[0:64, 10 + l : 11 + l],
                        )
                h = hn

            # ---- head: pack 4 tiles' [1,TT] outputs at partitions {0,32,64,96}
            for hg in range(TPM // 4):
                py = ps.tile([128, TT], f32, tag="po", bufs=1)
                for j in range(4):
                    t = hg * 4 + j
                    tsl = bass.ts(t, TT)
                    nc.tensor.matmul(
                        py[32 * j : 32 * j + 32, :], wout_sb, h[:, tsl],
                        start=True, stop=True, tile_position=(0, 32 * j),
                    )
                ysb = rstage.tile([128, TT], f32, tag="ysb")
                nc.vector.tensor_scalar_add(ysb, py, bcl_sb[:, 13:14])
                yrow = ysb.rearrange("(a b) f -> a b f", b=32)[:, 0, :]  # [4, TT]
                nc.sync.dma_start(
                    out=yv[mb * (TPM // 4) + hg : mb * (TPM // 4) + hg + 1, :],
                    in_=yrow,
                )

    nc.compile()
    return nc


def _prep_host_fast(inputs):
    f = np.float32
    x = np.asarray(inputs["x"], f).reshape(-1)
    t = np.asarray(inputs["t"], f).reshape(-1)
    W_in = np.asarray(inputs["W_in"], f)
    sW1 = np.asarray(inputs["sW1"], f)
    sW2 = np.asarray(inputs["sW2"], f)
    rW1 = np.asarray(inputs["rW1"], f)
    rW2 = np.asarray(inputs["rW2"], f)
    routW = np.asarray(inputs["routW"], f)
    W_out = np.asarray(inputs["W_out"], f)

    w1c = np.zeros((L, 128, 768), f)
    w2c = np.zeros((L, 128, 192), f)
    rtc = np.zeros((L, 128, 8), f)
    for l in range(L):
        # W1 stationaries zero-padded to K=128: (j, half) pairs; the odd
        # half's weight rows sit at 64:128 so the full-width h pairtile can
        # be the moving operand with no partition offsets.
        w1j = [np.transpose(sW1[l], (1, 0, 2)).reshape(64, 128),
               np.transpose(rW1[l, 0:2], (1, 0, 2)).reshape(64, 128),
               np.transpose(rW1[l, 2:4], (1, 0, 2)).reshape(64, 128)]
        for j in range(3):
            w1c[l, 0:64, (2 * j) * 128 : (2 * j + 1) * 128] = w1j[j]
            w1c[l, 64:128, (2 * j + 1) * 128 : (2 * j + 2) * 128] = w1j[j]
        w2c[l, :, 0:64] = sW2[l].reshape(128, 64)
        w2c[l, :, 64:128] = rW2[l, 0:2].reshape(128, 64)
        w2c[l, :, 128:192] = rW2[l, 2:4].reshape(128, 64)
        rtc[l, 0:64, 0:4] = routW[l]
        rtc[l, 64:128, 4:8] = routW[l]
    winc = np.zeros((2, 256), f)
    winc[:, 0:64] = W_in
    winc[:, 128 + 64 : 256] = W_in
    woutc = np.zeros((128, 512), f)
    wrep = np.repeat(W_out, 32, axis=1)  # [64, 32]
    for k in range(4):
        hb = 64 * (k % 2)
        woutc[hb : hb + 64, k * 128 + 32 * k : k * 128 + 32 * k + 32] = wrep

    import ml_dtypes
    shared = {
        "w1c": w1c, "w2c": w2c.astype(ml_dtypes.bfloat16), "rtc": rtc,
        "winc": winc, "woutc": woutc,
    }
    in_maps = []
    for c in range(NCORES):
        sl = slice(c * NPC, (c + 1) * NPC)
        xin = np.stack([x[sl], t[sl]], 0)  # [2, NPC]
        in_maps.append({"xin": np.ascontiguousarray(xin), **shared})
    return in_maps


def _prep_host(inputs):
    f = np.float32
    x = np.asarray(inputs["x"], f).reshape(-1)
    t = np.asarray(inputs["t"], f).reshape(-1)
    W_in = np.asarray(inputs["W_in"], f)
    b_in = np.asarray(inputs["b_in"], f)
    sW1 = np.asarray(inputs["sW1"], f)
    sb1 = np.asarray(inputs["sb1"], f)
    sW2 = np.asarray(inputs["sW2"], f)
    sb2 = np.asarray(inputs["sb2"], f)
    rW1 = np.asarray(inputs["rW1"], f)
    rb1 = np.asarray(inputs["rb1"], f)
    rW2 = np.asarray(inputs["rW2"], f)
    rb2 = np.asarray(inputs["rb2"], f)
    routW = np.asarray(inputs["routW"], f)
    routb = np.asarray(inputs["routb"], f)
    W_out = np.asarray(inputs["W_out"], f)
    b_out = np.asarray(inputs["b_out"], f)

    w1c = np.zeros((L, 64, 384), f)
    w2c = np.zeros((L, 128, 192), f)
    rtc = np.zeros((L, 64, 4), f)
    for l in range(L):
        w1c[l, :, 0:128] = np.transpose(sW1[l], (1, 0, 2)).reshape(64, 128)
        w1c[l, :, 128:256] = np.transpose(rW1[l, 0:2], (1, 0, 2)).reshape(64, 128)
        w1c[l, :, 256:384] = np.transpose(rW1[l, 2:4], (1, 0, 2)).reshape(64, 128)
        w2c[l, :, 0:64] = sW2[l].reshape(128, 64)
        w2c[l, :, 64:128] = rW2[l, 0:2].reshape(128, 64)
        w2c[l, :, 128:192] = rW2[l, 2:4].reshape(128, 64)
        rtc[l] = routW[l]
    rb2c = np.zeros((L, 2, 128, 64), f)
    for l in range(L):
        for half in range(2):
            for eb in range(2):
                e = half * 2 + eb
                rb2c[l, half, eb * 64 : (eb + 1) * 64, :] = rb2[l, e][None, :] / 64.0
    bcl = np.zeros((128, 14), f)
    bcl[0:64, 0] = b_in
    for l in range(L):
        bcl[:, 1 + 3 * l] = sb1[l].reshape(128)
        bcl[:, 2 + 3 * l] = rb1[l, 0:2].reshape(128)
        bcl[:, 3 + 3 * l] = rb1[l, 2:4].reshape(128)
        bcl[0:64, 10 + l] = sb2[l].sum(0)
    bcl[:, 13] = b_out[0]
    crw = np.zeros((128, 4 * L), f)
    for l in range(L):
        crw[:, 4 * l : 4 * l + 4] = np.exp(routb[l])[None, :]
    winc = np.ascontiguousarray(W_in)  # [2, 64]
    woutc = np.ascontiguousarray(np.repeat(W_out, 32, axis=1))  # [64, 32]
    idc = np.eye(128, dtype=f)

    shared = {
        "w1c": w1c, "w2c": w2c, "rtc": rtc, "rb2c": rb2c, "bcl": bcl,
        "crw": crw, "winc": winc, "woutc": woutc, "idc": idc,
    }
    in_maps = []
    for c in range(NCORES):
        sl = slice(c * NPC, (c + 1) * NPC)
        xin = np.stack([x[sl], t[sl]], 0)  # [2, NPC]
        in_maps.append({"xin": np.ascontiguousarray(xin), **shared})
    return in_maps


def _all_zero_biases(inputs):
    for k in ("b_in", "sb1", "rb1", "sb2", "rb2", "routb", "b_out"):
        if np.any(np.asarray(inputs[k]) != 0.0):
            return False
    return True


def _get_module(variant):
    if variant not in _CACHE:
        if variant == "fast":
            _CACHE[variant] = _build_fast_module()
        else:
            _CACHE[variant] = _build_module(with_rb2=(variant == "gen_rb2"))
    return _CACHE[variant]


def _run(inputs, trace=False):
    from concourse.bass_utils import run_bass_kernel_spmd

    if _all_zero_biases(inputs):
        in_maps = _prep_host_fast(inputs)
        nc = _get_module("fast")
    else:
        in_maps = _prep_host(inputs)
        with_rb2 = bool(np.any(np.asarray(inputs["rb2"]) != 0.0))
        nc = _get_module("gen_rb2" if with_rb2 else "gen")
    res = run_bass_kernel_spmd(
        nc, in_maps, core_ids=list(range(NCORES)), trace=trace
    )
    yy = np.concatenate([r["y"] for r in res.results], 0).astype(np.float32)
    return yy, res


def kernel(**inputs) -> np.ndarray:
    yy, _ = _run(inputs, trace=False)
    return yy
